# revision 13
# baseline (speedup 1.0000x reference)
"""MoE layer (B=4, N=2048, C=1024, F=4096, E=8, top-2) on 8 trn2 NeuronCores.

Sharding (fast path, b1 == b2 == 0 — the benchmarked case): F-parallel
over all experts.  The host computes the (tiny) router and builds ONE
gated, expert-major token stream shared by all cores; core d holds
f-blocks [4d, 4d+4) of EVERY expert's w1/w2 (same 16.8 MB SBUF footprint
as one full expert) and computes the partial FFN
    y_partial = relu(xg @ w1[fslice].T) @ w2[:, fslice].T
for ALL token-expert pairs.  The host sums the 8 partials per token.

Why F-parallel: every core runs the exact same instruction stream over
the exact same segment sizes (the true per-expert token counts padded
to 128), so per-core work is sum_e pad(n_e)/8 instead of max_e pad(n_e)
— the expert load imbalance vanishes instead of padding every core to
the hottest expert's count.  All matmuls in bf16 (1 cycle/row on the
PE); tokens pre-gated on host; y accumulates in fp32 PSUM across the 4
resident f-blocks of each chunk and retires once, in bf16.
"""

import numpy as np

P = 128
C = 1024
F = 4096
E = 8
NFB = 4  # f-blocks per core (32 total / 8 cores)
SCH = 384  # token chunk: 3 PSUM banks x 2 C-halves for y + 2 for h = 8


def _build(cap: int):
    """Slow fallback (nonzero biases): expert-parallel, fp32 weights."""
    import concourse.mybir as mybir
    from concourse import bacc
    from concourse.tile import TileContext

    f32 = mybir.dt.float32
    f32r = mybir.dt.float32r
    nS = cap // SCH
    nc = bacc.Bacc(None, target_bir_lowering=False)

    xgT = nc.dram_tensor("xgT", [C, cap], f32, kind="ExternalInput")
    w1t = nc.dram_tensor("w1t", [C, F], f32, kind="ExternalInput")
    w2t = nc.dram_tensor("w2t", [F, C], f32, kind="ExternalInput")
    b1r = nc.dram_tensor("b1r", [P, F // P], f32, kind="ExternalInput")
    b2r = nc.dram_tensor("b2r", [P, C], f32, kind="ExternalInput")
    wg = nc.dram_tensor("wg", [P, cap // P], f32, kind="ExternalInput")
    yg = nc.dram_tensor("yg", [cap, C], f32, kind="ExternalOutput")

    w1v = w1t.ap().rearrange("(co ci) f -> ci co f", ci=P)  # [128, 8, F]
    xgv = xgT.ap().rearrange("(co ci) n -> ci co n", ci=P)  # [128, 8, cap]

    with TileContext(nc) as tc:
        with (
            tc.tile_pool(name="consts", bufs=1) as consts,
            tc.tile_pool(name="wpool", bufs=4) as wpool,
            tc.tile_pool(name="xpool", bufs=2) as xpool,
            tc.tile_pool(name="hpool", bufs=3) as hpool,
            tc.tile_pool(name="ypool", bufs=3) as ypool,
            tc.tile_pool(name="psum_h", bufs=2, space="PSUM") as psum_h,
            tc.tile_pool(name="psum_y", bufs=1, space="PSUM") as psum_y,
        ):
            b1_sb = consts.tile([P, F // P], f32)
            nc.sync.dma_start(b1_sb[:], b1r[:, :])
            b2_sb = consts.tile([P, C], f32)
            nc.sync.dma_start(b2_sb[:], b2r[:, :])
            wg_sb = consts.tile([P, cap // P], f32)
            nc.sync.dma_start(wg_sb[:], wg[:, :])

            for s in range(nS):
                xg_s = xpool.tile([P, 8, SCH], f32r, tag="xg")
                nc.sync.dma_start(xg_s[:], xgv[:, :, s * SCH : (s + 1) * SCH].bitcast(f32r))

                yps = [
                    [
                        psum_y.tile(
                            [P, 512], f32, tag=f"y_{t}_{cc}", name=f"y_{t}_{cc}"
                        )
                        for cc in range(2)
                    ]
                    for t in range(3)
                ]

                for f in range(F // P):  # 32
                    w1c = wpool.tile([P, 8, P], f32r, tag="w1c")
                    nc.sync.dma_start(w1c[:], w1v[:, :, f * P : (f + 1) * P].bitcast(f32r))
                    w2c = wpool.tile([P, C], f32r, tag="w2c")
                    nc.sync.dma_start(w2c[:], w2t[f * P : (f + 1) * P, :].bitcast(f32r))

                    hps = psum_h.tile([P, SCH], f32, tag="h")
                    for c in range(8):
                        nc.tensor.matmul(
                            hps[:],
                            lhsT=w1c[:, c, :],
                            rhs=xg_s[:, c, :],
                            start=(c == 0),
                            stop=(c == 7),
                        )
                    hT = hpool.tile([P, SCH], f32r, tag="hT")
                    nc.scalar.activation(
                        hT[:],
                        hps[:],
                        mybir.ActivationFunctionType.Relu,
                        bias=b1_sb[:, f : f + 1],
                        scale=1.0,
                    )
                    for t in range(3):
                        for cc in range(2):
                            nc.tensor.matmul(
                                yps[t][cc][:],
                                lhsT=hT[:, t * P : (t + 1) * P],
                                rhs=w2c[:, cc * 512 : (cc + 1) * 512],
                                start=(f == 0),
                                stop=(f == F // P - 1),
                            )

                for t in range(3):
                    y_sb = ypool.tile([P, C], f32, tag="y_sb")
                    for cc in range(2):
                        sl = slice(cc * 512, (cc + 1) * 512)
                        nc.vector.tensor_add(y_sb[:, sl], yps[t][cc][:], b2_sb[:, sl])
                    yf = ypool.tile([P, C], f32, tag="yf")
                    nc.scalar.mul(yf[:], y_sb[:], wg_sb[:, s * 3 + t : s * 3 + t + 1])
                    nc.sync.dma_start(
                        yg[(s * 3 + t) * P : (s * 3 + t + 1) * P, :], yf[:]
                    )
    nc.compile()
    return nc


def _chunks(m, first=False):
    """Split a (128-multiple) segment into chunks of SCH with a runt last.

    first=True carves the leading chunk into [128, 256]: the kernel's very
    first matmul then only waits on a 262 KB token DMA instead of 786 KB,
    and the real matmuls (which double as HAM warm-up) start ~5 us sooner.
    Cost is one extra chunk's bookkeeping; mm1 columns and mm2 tile count
    are unchanged.
    """
    sizes = [SCH] * (m // SCH)
    rem = m - SCH * len(sizes)
    if rem:
        sizes.append(rem)
    if first and sizes and sizes[0] == SCH:
        sizes = [P, SCH - P] + sizes[1:]
    return sizes


def _build_fast(ms: tuple):
    """Fast path (b1 == 0 and b2 == 0): F-parallel over all experts.

    ms[e] = padded token count of expert e (multiple of 128; same on all
    cores).  Per core inputs:
      xgf [sum(ms)*C]        bf16 gated tokens, expert-major, per-chunk
                             [ci, co, n] tiles (identical on all cores)
      w1p [E, 4, 128, 8, 128] bf16 w1[e][fslice].T tiled for mm1 lhsT
      w2p [E, 4, 128, 1024]   bf16 w2[e][:, fslice].T tiled for mm2 rhs
    output:
      yg  [sum(ms), 1024] bf16 partial y (this core's f-slice term)

    All weights stay resident in SBUF (128 KB/partition); each token
    chunk's y accumulates in PSUM across the 4 f-blocks and retires once.
    """
    import concourse.mybir as mybir
    from concourse import bacc
    from concourse.tile import TileContext

    f32 = mybir.dt.float32
    bf16 = mybir.dt.bfloat16
    Mtot = sum(ms)
    nc = bacc.Bacc(None, target_bir_lowering=False)

    xgf = nc.dram_tensor("xgf", [Mtot * C], bf16, kind="ExternalInput")
    w1p = nc.dram_tensor("w1p", [E, NFB, P, 8, P], bf16, kind="ExternalInput")
    w2p = nc.dram_tensor("w2p", [E, NFB, P, C], bf16, kind="ExternalInput")
    yg = nc.dram_tensor("yg", [Mtot, C], bf16, kind="ExternalOutput")

    with TileContext(nc) as tc:
        with (
            tc.tile_pool(name="wpool", bufs=1) as wpool,
            tc.tile_pool(name="xpool", bufs=4) as xpool,
            tc.tile_pool(name="hpool", bufs=4) as hpool,
            tc.tile_pool(name="ypool", bufs=3) as ypool,
            tc.tile_pool(name="psum_h", bufs=2, space="PSUM") as psum_h,
            tc.tile_pool(name="psum_y", bufs=1, space="PSUM") as psum_y,
        ):
            # (expert, chunk) schedule, expert-major.  No dummy warm-up
            # block: the first chunk is deliberately tiny (128 tokens) so
            # real matmuls start as soon as its 262 KB lands (~3 us) and
            # themselves warm the HAM clock gate.
            sched = []  # (e, chunk_off_in_rows, size)
            offs = []
            off = 0
            for e in range(E):
                offs.append(off)
                coff = 0
                for sz in _chunks(ms[e], first=(e == 0)):
                    sched.append((e, off + coff, sz))
                    coff += sz
                off += ms[e]

            def load_xg(si):
                e, row0, sz = sched[si]
                xg_s = xpool.tile([P, 8, sz], bf16, tag="xg", name="xg_s")
                src = xgf[row0 * C : (row0 + sz) * C]
                v = src.rearrange("(ci co n) -> ci co n", ci=P, co=8)
                nc.sync.dma_start(xg_s[:], v)
                return xg_s

            w1g = wpool.tile([P, E, NFB, 8, P], bf16, tag="w1g", name="w1g")
            w2g = wpool.tile([P, E, NFB, C], bf16, tag="w2g", name="w2g")

            loaded = [False] * (E * NFB)  # (e, fl) weight pairs issued

            def load_wpair(e, fl):
                if not loaded[e * NFB + fl]:
                    loaded[e * NFB + fl] = True
                    nc.sync.dma_start(w1g[:, e, fl], w1p[e, fl])
                    nc.sync.dma_start(w2g[:, e, fl], w2p[e, fl])

            # DMA issue order matters: all loads drain through ONE in-order
            # hardware queue, so interleave token-chunk prefetches with
            # weight loads in consumption order.  Issuing all 16.8 MB of
            # weights up front starves the per-chunk token DMAs behind
            # them and stalls the PE for ~40 us.  Prologue: chunk 0's
            # 262 KB first, then expert 0's f-blocks interleaved with the
            # next chunks; afterwards expert e+1's four pairs trickle out
            # one per chunk during expert e's segment.
            xg_q = [load_xg(0)]
            load_wpair(0, 0)
            if len(sched) > 1:
                xg_q.append(load_xg(1))
            load_wpair(0, 1)
            if len(sched) > 2:
                xg_q.append(load_xg(2))
            load_wpair(0, 2)
            load_wpair(0, 3)
            PREF = 3  # xg prefetch depth (xpool bufs = PREF + 1)

            for si, (e, row0, sz) in enumerate(sched):
                if si == 0 or sched[si - 1][0] != e:
                    for fl in range(NFB):  # safety: must be resident now
                        load_wpair(e, fl)
                    seg_chunk = 0
                else:
                    seg_chunk += 1
                nt = (sz + P - 1) // P
                xg_s = xg_q.pop(0)
                if si + PREF < len(sched):
                    xg_q.append(load_xg(si + PREF))
                if e + 1 < E and seg_chunk < NFB:
                    load_wpair(e + 1, seg_chunk)

                yps = [
                    psum_y.tile([P, C], f32, tag=f"y_{t}", name=f"y_{t}")
                    for t in range(nt)
                ]

                final_chunk = si == len(sched) - 1

                def retire_tile(t, row0=row0, yps=yps, split=False):
                    r = row0 + t * P
                    if split:
                        # tail shave for the very last tile: retire each
                        # 512-col half as soon as ITS accumulation closes,
                        # on separate engines — halves the post-matmul tail
                        for cc in range(2):
                            sl = slice(cc * 512, (cc + 1) * 512)
                            yf = ypool.tile([P, 512], bf16, tag=f"yfs{cc}", name="yfs")
                            if cc == 0:
                                nc.vector.tensor_copy(yf[:], yps[t][:, sl])
                            else:
                                nc.scalar.activation(
                                    yf[:], yps[t][:, sl],
                                    mybir.ActivationFunctionType.Copy,
                                )
                            nc.sync.dma_start(yg[r : r + P, sl], yf[:])
                        return
                    # single retire per tile: PSUM -> bf16 SBUF -> DRAM,
                    # alternating DVE / ACT so neighbors drain in parallel
                    yf = ypool.tile([P, C], bf16, tag=f"yf{t % 2}", name="yf")
                    if t % 2 == 0:
                        nc.vector.tensor_copy(yf[:], yps[t][:])
                    else:
                        nc.scalar.activation(
                            yf[:], yps[t][:], mybir.ActivationFunctionType.Copy
                        )
                    nc.sync.dma_start(yg[r : r + P, :], yf[:])

                def mm2(fl, hT, last=False, yps=yps, nt=nt, e=e,
                        final_chunk=final_chunk):
                    for t in range(nt):
                        split = last and final_chunk and t == nt - 1
                        for cc in range(2):
                            nc.tensor.matmul(
                                yps[t][:, cc * 512 : (cc + 1) * 512],
                                lhsT=hT[:, t * P : (t + 1) * P],
                                rhs=w2g[:, e, fl, cc * 512 : (cc + 1) * 512],
                                start=(fl == 0),
                                stop=(fl == NFB - 1),
                            )
                        if last:
                            # retire as soon as this tile's accumulation
                            # closes: frees its PSUM banks for the next
                            # chunk's mm2 that much earlier
                            retire_tile(t, split=split)

                # software pipeline: mm2 runs two fl behind mm1, so the
                # relu feeding each mm2 block retired long before the PE
                # reaches it; the last two mm2 blocks + the PSUM retires
                # drain after the next chunk's first mm1 blocks
                hTs = []
                for fl in range(NFB):
                    hps = psum_h.tile([P, SCH], f32, tag="h", name="hps")
                    for c in range(8):
                        nc.tensor.matmul(
                            hps[:, :sz],
                            lhsT=w1g[:, e, fl, c, :],
                            rhs=xg_s[:, c, :],
                            start=(c == 0),
                            stop=(c == 7),
                        )
                    hT = hpool.tile([P, SCH], bf16, tag="hT", name="hT")
                    if fl >= NFB - 2:
                        # last fl's: per-token-tile relu so mm2(t) can
                        # start as soon as its slice is ready
                        for t in range(nt):
                            tl = slice(t * P, min((t + 1) * P, sz))
                            nc.scalar.activation(
                                hT[:, tl],
                                hps[:, tl],
                                mybir.ActivationFunctionType.Relu,
                            )
                    else:
                        nc.scalar.activation(
                            hT[:, :sz],
                            hps[:, :sz],
                            mybir.ActivationFunctionType.Relu,
                        )
                    hTs.append(hT)
                    if fl >= 2:
                        mm2(fl - 2, hTs[fl - 2])
                mm2(NFB - 2, hTs[NFB - 2])
                mm2(NFB - 1, hTs[NFB - 1], last=True)
    nc.compile()
    return nc


_CACHE = {}
_TRACE = False  # test harness sets True to capture an NTFF profile
_LAST_RES = None


def _get_nc(key, builder):
    if key not in _CACHE:
        _CACHE[key] = builder()
    return _CACHE[key]


def _route(x_flat, router_w):
    """Top-2 routing, float64 for stable selection. Returns idx/weights per expert."""
    logits = x_flat.astype(np.float64) @ router_w.astype(np.float64).T
    t = np.exp(logits - logits.max(-1, keepdims=True))
    p = t / t.sum(-1, keepdims=True)
    top2 = np.argsort(-p, axis=-1)[:, :2]
    pv = np.take_along_axis(p, top2, axis=-1)
    wn = pv / (pv.sum(-1, keepdims=True) + 1e-9)
    return top2, wn


def kernel(x, router_w, w1, b1, w2, b2):
    import ml_dtypes
    from concourse.bass_utils import run_bass_kernel_spmd

    bf16 = ml_dtypes.bfloat16
    Bx, Nx, Cx = x.shape
    x_flat = np.ascontiguousarray(x.reshape(-1, Cx))
    T = x_flat.shape[0]

    top2, wn = _route(x_flat, router_w)
    idxs, gates = [], []
    for e in range(E):
        sel = top2 == e
        we = np.where(sel, wn, 0.0).sum(-1)
        idx = np.nonzero(sel.any(-1))[0]
        idxs.append(idx)
        gates.append(we[idx].astype(np.float32))

    fast = bool(np.all(b1 == 0) and np.all(b2 == 0))
    global _LAST_RES

    if not fast:
        cap = max(len(i) for i in idxs)
        cap = ((cap + SCH - 1) // SCH) * SCH
        nc = _get_nc(("slow", cap), lambda: _build(cap))
        in_maps = []
        for e in range(E):
            n_e = len(idxs[e])
            xg = np.zeros((cap, Cx), np.float32)
            xg[:n_e] = x_flat[idxs[e]]
            wg = np.zeros(cap, np.float32)
            wg[:n_e] = gates[e]
            in_maps.append(
                {
                    "xgT": np.ascontiguousarray(xg.T),
                    "w1t": np.ascontiguousarray(w1[e].T),
                    "w2t": np.ascontiguousarray(w2[e].T),
                    "b1r": np.ascontiguousarray(b1[e].reshape(F // P, P).T),
                    "b2r": np.ascontiguousarray(np.broadcast_to(b2[e], (P, Cx))),
                    "wg": np.ascontiguousarray(wg.reshape(cap // P, P).T),
                }
            )
        res = run_bass_kernel_spmd(nc, in_maps, core_ids=list(range(E)), trace=_TRACE)
        _LAST_RES = res
        out = np.zeros((T, Cx), np.float32)
        for e in range(E):
            n_e = len(idxs[e])
            out[idxs[e]] += res.results[e]["yg"][:n_e].astype(np.float32)
        return out.reshape(Bx, Nx, Cx)

    # ---- fast path: F-parallel over all experts ----
    # order experts so the very last chunk is as small as possible (the
    # final retire + output DMA is the kernel's tail)
    ms_nat = [((len(idxs[e]) + P - 1) // P) * P for e in range(E)]
    order = sorted(range(E), key=lambda e: (-(ms_nat[e] % SCH or SCH), e))
    ms = tuple(ms_nat[e] for e in order)
    nc = _get_nc(("fast", ms), lambda: _build_fast(ms))

    # shared gated token stream, expert-major, per-chunk [ci, co, n] tiles
    blocks = []
    for i, e in enumerate(order):
        n_e = len(idxs[e])
        m_e = ms_nat[e]
        xg = np.zeros((m_e, Cx), np.float32)
        xg[:n_e] = x_flat[idxs[e]] * gates[e][:, None]  # pre-gate (b1 == 0)
        xgb = xg.astype(bf16)
        off = 0
        for sz in _chunks(m_e, first=(i == 0)):
            blocks.append(
                np.ascontiguousarray(
                    xgb[off : off + sz].reshape(sz, 8, P).transpose(2, 1, 0)
                ).ravel()
            )
            off += sz
    xgf = np.concatenate(blocks)

    # per-core weight slices: core d holds f-blocks [4d, 4d+4) of every expert
    # w1 tiled:  w1t[e][fb, fo, c, ci] -> lhsT layout [ci, c, fo]
    w1t = [
        w1[e].reshape(F // P, P, 8, P).transpose(0, 3, 2, 1).astype(bf16)
        for e in order
    ]
    w2t = [w2[e].T.reshape(F // P, P, Cx).astype(bf16) for e in order]
    in_maps = []
    for d in range(8):
        fsl = slice(NFB * d, NFB * (d + 1))
        w1pd = np.ascontiguousarray(np.stack([w1t[i][fsl] for i in range(E)]))
        w2pd = np.ascontiguousarray(np.stack([w2t[i][fsl] for i in range(E)]))
        in_maps.append({"xgf": xgf, "w1p": w1pd, "w2p": w2pd})

    res = run_bass_kernel_spmd(nc, in_maps, core_ids=list(range(8)), trace=_TRACE)
    _LAST_RES = res

    # host combine: sum the 8 partial-y streams, then scatter-add per expert
    ysum = res.results[0]["yg"].astype(np.float32)
    for d in range(1, 8):
        ysum += res.results[d]["yg"].astype(np.float32)
    out = np.zeros((T, Cx), np.float32)
    off = 0
    for i, e in enumerate(order):
        n_e = len(idxs[e])
        out[idxs[e]] += ysum[off : off + n_e]
        off += ms[i]
    return out.reshape(Bx, Nx, Cx)
